# revision 40
# baseline (speedup 1.0000x reference)
"""Causal self-attention (B=4, T=2048, C=1024, H=16, D=64) on 8 trn2 NeuronCores.

Sharding: core = 2*b + g  (b = batch 0..3, g = head-group 0..1, 8 heads each).
Data parallel over B, tensor parallel over heads; each core computes a partial
out-projection (its 512 y-channels x out_w columns) and the host sums the two
partials per batch (the tensor-parallel all-reduce) and adds biases.

Per-core device program (all matmuls bf16 in / f32 PSUM accumulate):
  qkT[ch, t] = (Wqk x^T)       - q rows pre-scaled by 1/sqrt(D) host-side
  v[t, ch]   = x Wv^T          - v bias folded into the final host bias add
  S^T[kt,qt] = kT . q          - per head; causal via N-slicing + tri mask
  P = exp(S^T)                 - no max subtraction (|S| < 4, exact-safe)
  O^T,l      = [v|1]^T P       - ones column rides the PV matmul => l = sum P
  yT         = O^T * (1/l)     - 1/l = exp(-ln l) on ScalarE, DMA broadcast
  out_part   = yT^T Wo_part^T  - partial over this core's 512 channels
"""

import os
import numpy as np
import ml_dtypes
from contextlib import ExitStack

B, T, C, H, D = 4, 2048, 1024, 16, 64
P = 128
N_CORES = 8
HPG = H // 2          # heads per group/core = 8
GC = HPG * D          # channels per group = 512
BF16 = ml_dtypes.bfloat16

_BUILT = {}
# pool-buffer config; PSUM constraint: mm + s + o <= 8 banks
_CFG = {"mm": 2, "s": 3, "o": 3, "p": 12, "n": 6, "out": 6}
_TRI = "gpsimd"          # which engine applies the diagonal tri mask
_TRI_ENGINE = lambda nc: getattr(nc, _TRI)
_ABLATE_EXP = False      # timing ablation only: skip exp+mask
_QC_ORDER = (3, 2, 1, 0)   # big chunks first: deeper pipeline, short tail
_STAGEA_T4_OUTER = False
_NORM_DMA_ENGINE = "sync"   # queue for the small normalize-chain DMAs
_NORM_DMA = lambda nc: getattr(nc, _NORM_DMA_ENGINE)


def _split_multiwait_sync(nc):
    """This container's walrus rejects instructions carrying more than one
    sync-wait command ("Too many sync wait commands", setupSyncWait). Tile's
    scheduler emits such instructions (e.g. the end-of-context drain waits on
    every DMA-queue semaphore at once). Split them: hoist all but the last
    wait onto single-wait Drain instructions inserted just before, on the
    same engine — semantically identical (engine stalls on each in turn)."""
    import bass_rust
    from concourse import mybir

    n = 0
    for func in nc.m.functions:
        for block in func.blocks:
            insts = list(block.instructions)
            out = []
            changed = False
            for inst in insts:
                si = inst.sync_info
                waits = list(si.on_wait) if si is not None and si.on_wait else []
                if len(waits) > 1:
                    changed = True
                    for w in waits[:-1]:
                        d = mybir.InstDrain(
                            name=f"{inst.name}_swait{n}", ins=[], outs=[])
                        n += 1
                        d.engine = inst.engine
                        d.sync_info = bass_rust.SyncInfo(
                            on_wait=[w], on_update=[])
                        out.append(d)
                    si.on_wait = [waits[-1]]
                    inst.sync_info = si
                out.append(inst)
            if changed:
                block.instructions = out


def _build_bass(reps=1):
    """Build the (core-uniform) Bass program once per process.

    reps > 1 emits the whole body N times inside one NEFF — used only by the
    timing harness to amortize the multi-ms per-dispatch overhead of this
    axon client (NTFF profiling is unavailable here)."""
    key = ("nc", reps)
    if key in _BUILT:
        return _BUILT[key]

    import concourse.bass as bass
    import concourse.tile as tile
    from concourse import mybir

    DT = mybir.dt.bfloat16
    F32 = mybir.dt.float32
    EXP = mybir.ActivationFunctionType.Exp
    LN = mybir.ActivationFunctionType.Ln
    IDENT = mybir.ActivationFunctionType.Identity
    MUL = mybir.AluOpType.mult

    nc = bass.Bass("TRN2", target_bir_lowering=False, debug=False)

    xT_d = nc.dram_tensor("xT", [P, 8, T], DT, kind="ExternalInput").ap()
    wqk_d = nc.dram_tensor("wqk", [P, 8, 1024], DT, kind="ExternalInput").ap()
    wv_d = nc.dram_tensor("wv", [P, 8, GC], DT, kind="ExternalInput").ap()
    bqk_d = nc.dram_tensor("bqk", [P, 8], F32, kind="ExternalInput").ap()
    wo_d = nc.dram_tensor("wo", [P, 4, 1024], DT, kind="ExternalInput").ap()
    tri_d = nc.dram_tensor("tri", [P, P], DT, kind="ExternalInput").ap()
    out_d = nc.dram_tensor("out", [T, 1024], F32, kind="ExternalOutput").ap()

    cfg = _CFG
    with tile.TileContext(nc) as tc, ExitStack() as ctx:
        consts = ctx.enter_context(tc.tile_pool(name="consts", bufs=1))
        ppool = ctx.enter_context(tc.tile_pool(name="ppool", bufs=cfg["p"]))
        npool = ctx.enter_context(tc.tile_pool(name="npool", bufs=cfg["n"]))
        outp = ctx.enter_context(tc.tile_pool(name="outp", bufs=cfg.get("out", 3)))
        mmps = ctx.enter_context(
            tc.tile_pool(name="mmps", bufs=cfg["mm"], space="PSUM"))
        sps = ctx.enter_context(
            tc.tile_pool(name="sps", bufs=cfg["s"], space="PSUM"))
        ops = ctx.enter_context(
            tc.tile_pool(name="ops", bufs=cfg["o"], space="PSUM"))

        for rep in range(reps):
            _emit_body(nc, tc, consts, ppool, npool, outp, mmps, sps, ops,
                       xT_d, wqk_d, wv_d, bqk_d, wo_d, tri_d, out_d, rep)

    _split_multiwait_sync(nc)
    _BUILT[key] = nc
    return nc


def _emit_body(nc, tc, consts, ppool, npool, outp, mmps, sps, ops,
               xT_d, wqk_d, wv_d, bqk_d, wo_d, tri_d, out_d, rep):
    from concourse import mybir
    DT = mybir.dt.bfloat16
    F32 = mybir.dt.float32
    EXP = mybir.ActivationFunctionType.Exp
    IDENT = mybir.ActivationFunctionType.Identity
    MUL = mybir.AluOpType.mult
    # --- persistent SBUF tensors (same tag across reps -> shared slots) ---
    xT = consts.tile([P, 8, T], DT, tag="xT", name=f"xT{rep}")
    wqk = consts.tile([P, 8, 1024], DT, tag="wqk", name=f"wqk{rep}")
    wv = consts.tile([P, 8, GC], DT, tag="wv", name=f"wv{rep}")
    bqk = consts.tile([P, 8], F32, tag="bqk", name=f"bqk{rep}")
    wo = consts.tile([P, 4, 1024], DT, tag="wo", name=f"wo{rep}")
    tri = consts.tile([P, P], DT, tag="tri", name=f"tri{rep}")
    qkT = consts.tile([P, 8, T], DT, tag="qkT", name=f"qkT{rep}")
    v = consts.tile([P, 16, HPG, 65], DT, tag="v", name=f"v{rep}")
    yT = consts.tile([P, 4, T], DT, tag="yT", name=f"yT{rep}")

    for ki in range(8):
        nc.sync.dma_start(xT[:, ki, :], xT_d[:, ki, :])
        nc.sync.dma_start(wqk[:, ki, :], wqk_d[:, ki, :])
        nc.sync.dma_start(wv[:, ki, :], wv_d[:, ki, :])
    for kc in range(4):
        nc.sync.dma_start(wo[:, kc, :], wo_d[:, kc, :])
    nc.sync.dma_start(bqk[:], bqk_d[:])
    nc.sync.dma_start(tri[:], tri_d[:])

    # ones column for the PV-matmul row that accumulates l = sum(P)
    nc.vector.memset(v[:, :, :, 64], 1.0)      # every head: [v | 1]

    # --- stage A: projections ---
    def qk_group(mch, t4):
        ps = mmps.tile([P, 512], F32, tag="mm", name=f"mmq{rep}_{mch}_{t4}")
        for ki in range(8):
            nc.tensor.matmul(
                ps[:],
                lhsT=wqk[:, ki, mch * 128:(mch + 1) * 128],
                rhs=xT[:, ki, t4 * 512:(t4 + 1) * 512],
                start=(ki == 0), stop=(ki == 7),
            )
        # bias in-place in PSUM, then split the PSUM->SBUF copy between
        # DVE and ScalarE Copy (resident in every ACT table set) so the
        # DVE 1x copy doesn't pace stage A.
        nc.vector.tensor_scalar_add(ps[:], ps[:], bqk[:, mch:mch + 1])
        nc.vector.tensor_copy(
            qkT[:, mch, t4 * 512:t4 * 512 + 256], ps[:, 0:256])
        nc.scalar.copy(
            qkT[:, mch, t4 * 512 + 256:(t4 + 1) * 512], ps[:, 256:512])

    def v_group(mt):
        ps = mmps.tile([P, 512], F32, tag="mm", name=f"mmv{rep}_{mt}")
        for ki in range(8):
            nc.tensor.matmul(
                ps[:],
                lhsT=xT[:, ki, mt * 128:(mt + 1) * 128],
                rhs=wv[:, ki, :],
                start=(ki == 0), stop=(ki == 7),
            )
        psr = ps[:].rearrange("p (h d) -> p h d", h=HPG)
        nc.vector.tensor_copy(v[:, mt, 0:4, 0:64], psr[:, 0:4, :])
        nc.scalar.copy(v[:, mt, 4:8, 0:64], psr[:, 4:8, :])

    if _STAGEA_T4_OUTER:
        # t4-outer: the t-chunks the first attention chunk needs come first
        for t4 in range(4):
            for mch in range(8):
                qk_group(mch, t4)
            if t4 == 0:
                for mt in range(4):
                    v_group(mt)
        for mt in range(4, 16):
            v_group(mt)
    else:
        for mch in range(8):
            for t4 in range(4):
                qk_group(mch, t4)
        for mt in range(16):
            v_group(mt)

    # --- stage B: attention, head-pairs packed on PE row groups;
    #     stage C (out-proj) interleaved per finished q-chunk ---
    for qc in _QC_ORDER:
        for hp in range(4):
            nkt = 4 * (qc + 1)
            po = [ops.tile([P, 512], F32, tag="po", name=f"po{rep}_{hp}_{qc}_{i}")
                  for i in range(2)]
            for ki in range(nkt):
                j = ki - 4 * qc
                n0 = 128 * j if j >= 0 else 0
                for hh in range(2):
                    h = 2 * hp + hh
                    pb = hh * 64
                    ps = sps.tile([P, 512], F32, tag="s",
                                  name=f"s{rep}_{hp}_{qc}_{ki}_{hh}")
                    nc.tensor.matmul(
                        ps[:, n0:512],
                        lhsT=qkT[pb:pb + 64, 4 + hp, ki * 128:(ki + 1) * 128],
                        rhs=qkT[pb:pb + 64, hp, qc * 512 + n0:(qc + 1) * 512],
                        start=True, stop=True,
                    )
                    pt = ppool.tile([P, 512], DT, tag="p",
                                    name=f"p{rep}_{hp}_{qc}_{ki}_{hh}")
                    if not _ABLATE_EXP:
                        nc.scalar.activation(pt[:, n0:512], ps[:, n0:512], EXP)
                        if j >= 0:
                            _TRI_ENGINE(nc).tensor_tensor(
                                pt[:, n0:n0 + 128], pt[:, n0:n0 + 128],
                                tri[:], MUL)
                    else:
                        nc.vector.memset(pt[0:1, 0:1], 0.0)
                    nc.tensor.matmul(
                        po[hh][0:65, n0:512],
                        lhsT=v[:, ki, h, :],
                        rhs=pt[:, n0:512],
                        start=(ki == 0), stop=(ki == nkt - 1),
                    )
            for hh in range(2):
                # O rows at partitions 0-63, l = sum(P) at partition 64.
                # 1/l: l-row -> SBUF (DVE), spread [1,512]->[64,8] (DMA),
                # reciprocal on 64 lanes (DVE, 8 elems each), gather back,
                # then partition-broadcast via 0-stride-source DMA. The
                # direct 1-partition reciprocal would be 4.3us/call and the
                # gpsimd/custom-DVE fast paths don't exist in this walrus.
                lsb = npool.tile([P, 520], F32, tag="l",
                                 name=f"l{rep}_{hp}_{qc}_{hh}")
                nc.vector.tensor_copy(lsb[64:65, 0:512], po[hh][64:65, :])
                _NORM_DMA(nc).dma_start(
                    lsb[0:64, 512:520],
                    lsb[64:65, 0:512].rearrange("o (p e) -> o p e", p=64))
                nc.vector.reciprocal(lsb[0:64, 512:520], lsb[0:64, 512:520])
                _NORM_DMA(nc).dma_start(lsb[64:65, 0:512], lsb[0:64, 512:520])
                rb = npool.tile([P, 512], F32, tag="rb",
                                name=f"rb{rep}_{hp}_{qc}_{hh}")
                _NORM_DMA(nc).dma_start(
                    rb[0:64, :],
                    lsb[64:65, None, 0:512].to_broadcast((1, 64, 512)))
                if hh == 0:
                    nc.vector.tensor_tensor(
                        yT[0:64, hp, qc * 512:(qc + 1) * 512],
                        po[hh][0:64, :], rb[0:64, :], MUL)
                else:
                    # DVE is partition-aligned; normalize at 0-63 then
                    # DMA-shift the tile to partitions 64-127 of yT.
                    tmp = npool.tile([64, 512], DT, tag="tmp",
                                     name=f"tmp{rep}_{hp}_{qc}")
                    nc.vector.tensor_tensor(
                        tmp[:], po[hh][0:64, :], rb[0:64, :], MUL)
                    nc.sync.dma_start(
                        yT[64:128, hp, qc * 512:(qc + 1) * 512], tmp[:])

        # --- stage C: out-proj for the t-range this q-chunk completed ---
        for mt in range(4 * qc, 4 * qc + 4):
            for n2 in range(2):
                ps = mmps.tile([P, 512], F32, tag="mm",
                               name=f"mmo{rep}_{mt}_{n2}")
                for kc in range(4):
                    nc.tensor.matmul(
                        ps[:],
                        lhsT=yT[:, kc, mt * 128:(mt + 1) * 128],
                        rhs=wo[:, kc, n2 * 512:(n2 + 1) * 512],
                        start=(kc == 0), stop=(kc == 3),
                    )
                osb = outp.tile([P, 512], F32, tag="o", name=f"o{rep}_{mt}_{n2}")
                nc.vector.tensor_copy(osb[:], ps[:])
                nc.sync.dma_start(
                    out_d[mt * 128:(mt + 1) * 128, n2 * 512:(n2 + 1) * 512],
                    osb[:])


def _ktiled(a, np_dtype):
    """[C_in, N] -> [128, C_in//128, N] (contraction partition-tiled)."""
    cin, n = a.shape
    return np.ascontiguousarray(
        a.reshape(cin // P, P, n).transpose(1, 0, 2)).astype(np_dtype)


def _make_in_maps(x, qkv_w, qkv_b, out_w, out_b):
    scale = 1.0 / np.sqrt(D)
    qw = qkv_w[0:C].reshape(H, D, C)
    kw = qkv_w[C:2 * C].reshape(H, D, C)
    vw = qkv_w[2 * C:3 * C].reshape(H, D, C)
    qb = qkv_b[0:C].reshape(H, D)
    kb = qkv_b[C:2 * C].reshape(H, D)

    tri = np.where(np.arange(P)[None, :] >= np.arange(P)[:, None],
                   np.float32(1), np.float32(0)).astype(BF16)

    xT_b = [_ktiled(np.ascontiguousarray(x[b].T), BF16) for b in range(B)]

    grp = []
    for g in range(2):
        hs = slice(g * HPG, (g + 1) * HPG)
        wqk_g = np.concatenate(
            [qw[hs].reshape(GC, C) * scale, kw[hs].reshape(GC, C)], 0)
        bqk_g = np.concatenate(
            [qb[hs].reshape(GC) * scale, kb[hs].reshape(GC)], 0)
        wv_g = vw[hs].reshape(GC, C)
        wo_g = out_w[:, g * GC:(g + 1) * GC]    # [1024, 512]
        grp.append({
            "wqk": _ktiled(np.ascontiguousarray(wqk_g.T), BF16),
            "bqk": np.ascontiguousarray(bqk_g.reshape(8, P).T).astype(np.float32),
            "wv": _ktiled(np.ascontiguousarray(wv_g.T), BF16),
            "wo": _ktiled(np.ascontiguousarray(wo_g.T), BF16),
        })

    in_maps = []
    for core in range(N_CORES):
        b, g = core // 2, core % 2
        in_maps.append({
            "xT": xT_b[b],
            "wqk": grp[g]["wqk"],
            "wv": grp[g]["wv"],
            "bqk": grp[g]["bqk"],
            "wo": grp[g]["wo"],
            "tri": tri,
        })
    return in_maps


def kernel(x, qkv_w, qkv_b, out_w, out_b):
    from concourse.bass_utils import run_bass_kernel_spmd

    x = np.asarray(x, np.float32)
    qkv_w = np.asarray(qkv_w, np.float32)
    qkv_b = np.asarray(qkv_b, np.float32)
    out_w = np.asarray(out_w, np.float32)
    out_b = np.asarray(out_b, np.float32)

    nc = _build_bass()
    in_maps = _make_in_maps(x, qkv_w, qkv_b, out_w, out_b)

    res = run_bass_kernel_spmd(
        nc, in_maps, core_ids=list(range(N_CORES)), trace=False)
    _BUILT["last_exec_time_ns"] = res.exec_time_ns
    _BUILT["in_maps"] = in_maps

    # host-side unshard: tensor-parallel all-reduce + all folded biases
    bias_full = out_b + out_w @ qkv_b[2 * C:3 * C]
    out = np.empty((B, T, C), np.float32)
    for b in range(B):
        out[b] = (np.asarray(res.results[2 * b]["out"])
                  + np.asarray(res.results[2 * b + 1]["out"])
                  + bias_full[None, :])
    return out
